# revision 18
# baseline (speedup 1.0000x reference)
"""Trainium2 Bass kernel for nn_CausalAttentionKVCache (B=2, T=2048, D=1024, 16 heads).

Sharding: 8 cores = 2 batch-halves x 4 head-groups (4 heads each).
Two compiled SPMD programs (one per batch-half, phase constants differ mod 3),
dispatched concurrently on jax devices [0:4] and [4:8].

The module's reshape y.view(3,B,T,hs,nh) scrambles tokens: flat row
v = (c*B*T + b*T + t)//3 of y=[x@W+b] in column block j=(c*B*T+b*T+t)%3 holds
token t of tensor c (q/k/v). With a host-side column permutation of W
(W2[:, j*1024+h*64+d] = W[:, j*1024+d*16+h]) each head's 64 features are
contiguous, and each token-residue class (t mod 3) is a contiguous row-run.

Per core: YT_qk = WQK^T @ xT (features on partitions) feeds Q^T (strided
descramble on PSUM eviction) and K^T (contiguous, v-indexed); V is projected
per-residue with a shifted v-window so its rows align with the k-chunk grid,
with a ones-column per head for the softmax denominator. Attention runs in
S^T = K^T.T@Q^T layout (k on partitions): exp on ScalarE (scale=1/8 fused, no
max-subtraction needed: scores ~ N(0,1)), causal staircase zeroed by gpsimd
affine_select, PV with V stationary accumulating ctx^T, PE-transpose + per-
partition reciprocal for the final division. Matmuls use float32r
(~1.5e-4 rel err, 4x fp32 throughput).
"""
import sys
import os

sys.path.insert(0, "/opt/trn_rl_repo")

import numpy as np

import concourse.bass as bass
import concourse.bacc as bacc
import concourse.mybir as mybir
import concourse.tile as tile
from concourse.masks import make_identity

B, T, D, NH, HS = 2, 2048, 1024, 16, 64
NV = 684          # v-rows per (c, batch-half) slice
NVV = 772         # XTV slice width (guard col + 768 window + pad)
GUARD = 1
NCHUNK = 6        # k/v chunks of 128 rows
QW = 1024         # q window
F32R = mybir.dt.float32r
F32 = mybir.dt.float32

_CACHE = {}


def _phase(B2):
    """Compile-time residue/offset constants for batch-half B2."""
    cst = {}
    for c in range(3):
        u0 = c * B * T + B2 * T
        vstart = u0 // 3
        rc_of_jj, r0_of_jj = {}, {}
        for rc in range(3):
            jj = (u0 + rc) % 3
            rc_of_jj[jj] = rc
            r0_of_jj[jj] = (u0 + rc - jj) // 3 - vstart
        cst[c] = dict(u0=u0, vstart=vstart, rc=rc_of_jj, r0=r0_of_jj)
    # rc-indexed views
    jk = {cst[1]["rc"][j]: j for j in range(3)}
    r0k = {cst[1]["rc"][j]: cst[1]["r0"][j] for j in range(3)}
    jv = {cst[2]["rc"][j]: j for j in range(3)}
    r0v = {cst[2]["rc"][j]: cst[2]["r0"][j] for j in range(3)}
    return cst, jk, r0k, jv, r0v


def _chunks(B2, q0):
    """Valid k-chunks (m, rc) for q-window [q0, q0+QW), with extents."""
    _, jk, r0k, _, _ = _phase(B2)
    out = []
    for m in range(NCHUNK):
        for rc in range(3):
            t_min = rc + 3 * (128 * m - r0k[rc])
            if t_min >= q0 + QW:
                continue
            a = max(0, t_min - q0) & ~1
            out.append((m, rc, t_min, a))
    return out


def _build_program(B2, repeat=1):
    cst, jk, r0k, jv, r0v = _phase(B2)
    nc = bacc.Bacc("TRN2", target_bir_lowering=False, debug=False, num_devices=4)

    xtq_d = nc.dram_tensor("XTQ", [D, 768], F32R, kind="ExternalInput")
    xtk_d = nc.dram_tensor("XTK", [D, 768], F32R, kind="ExternalInput")
    xtv_d = nc.dram_tensor("XTV", [D, NVV], F32R, kind="ExternalInput")
    wqk_d = nc.dram_tensor("WQK", [D, 768], F32R, kind="ExternalInput")
    wv_d = nc.dram_tensor("WV", [D, 780], F32R, kind="ExternalInput")
    bqk_d = nc.dram_tensor("BQK", [128, 6], F32, kind="ExternalInput")
    bv_d = nc.dram_tensor("BV", [1, 780], F32R, kind="ExternalInput")
    ones_d = nc.dram_tensor("ONES", [1, 128], F32R, kind="ExternalInput")
    zeros_d = nc.dram_tensor("ZEROS", [128, 260], F32R, kind="ExternalInput")
    out_d = nc.dram_tensor("OUT", [T, 256], F32, kind="ExternalOutput")

    from contextlib import ExitStack

    with tile.TileContext(nc) as tc:
        with (
            tc.tile_pool(name="const", bufs=1) as constp,
            tc.tile_pool(name="wpool", bufs=1) as wpool,
            tc.tile_pool(name="xpool", bufs=2) as xpool,
            tc.tile_pool(name="qkv", bufs=1) as qkvp,
            tc.tile_pool(name="ppool", bufs=2) as ppool,
            tc.tile_pool(name="cxpool", bufs=2) as cxpool,
            tc.tile_pool(name="opool", bufs=2) as opool,
            tc.tile_pool(name="rpool", bufs=2) as rpool,
        ):
            identity = constp.tile([128, 128], F32)
            make_identity(nc, identity[:])
            ones = constp.tile([1, 128], F32R)
            nc.sync.dma_start(ones[:], ones_d[:, :])
            zeros = constp.tile([128, 260], F32R)
            nc.sync.dma_start(zeros[:], zeros_d[:, :])

            wqk = wpool.tile([128, 8, 768], F32R)
            wv = wpool.tile([128, 8, 780], F32R)
            bqk = wpool.tile([128, 6], F32)
            bv = wpool.tile([1, 780], F32R)
            nc.sync.dma_start(wqk[:], wqk_d.rearrange("(c p) f -> p c f", p=128))
            nc.sync.dma_start(wv[:], wv_d.rearrange("(c p) f -> p c f", p=128))
            nc.sync.dma_start(bqk[:], bqk_d[:, :])
            nc.sync.dma_start(bv[:], bv_d[:, :])

            for _rep in range(repeat):
                qt = qkvp.tile([128, 2, T], F32R, tag="qt")
                kt = qkvp.tile([128, 2, 3, 768], F32R, tag="kt")
                yvs = qkvp.tile([128, NCHUNK, 3, 260], F32R, tag="yvs")

                proj_ctx = ExitStack()
                psqk = proj_ctx.enter_context(
                    tc.tile_pool(name="psqk", bufs=2, space="PSUM"))
                psv = proj_ctx.enter_context(
                    tc.tile_pool(name="psv", bufs=2, space="PSUM"))

                # ---------------- projections: Q^T, K^T ----------------
                for si, (src_d, name) in enumerate([(xtq_d, "xq"), (xtk_d, "xk")]):
                    xt = xpool.tile([128, 8, NVV], F32R, tag="xt", name=name)
                    nc.sync.dma_start(
                        xt[:, :, :768], src_d.rearrange("(c p) v -> p c v", p=128)
                    )
                    for fc in range(6):
                        ps = psqk.tile([128, 768], F32, tag="psqk")
                        for v0, v1 in ((0, 512), (512, 768)):
                            for ic in range(8):
                                nc.tensor.matmul(
                                    ps[:, v0:v1],
                                    wqk[:, ic, fc * 128:(fc + 1) * 128],
                                    xt[:, ic, v0:v1],
                                    start=(ic == 0),
                                    stop=(ic == 7),
                                )
                        jj, hp = fc // 2, fc % 2
                        if si == 0:   # Q: strided descramble eviction + bias
                            rc, r0 = cst[0]["rc"][jj], cst[0]["r0"][jj]
                            nrc = 683 if rc < 2 else 682
                            vlo, vhi = r0, min(NV, r0 + nrc)
                            n = vhi - vlo
                            nc.vector.tensor_scalar_add(
                                qt[:, hp, rc: min(rc + 3 * n, T): 3],
                                ps[:, vlo:vhi],
                                bqk[:, fc: fc + 1],
                            )
                        else:         # K: contiguous, v-indexed
                            nc.vector.tensor_scalar_add(
                                kt[:, hp, jj, 0:NV],
                                ps[:, 0:NV],
                                bqk[:, fc: fc + 1],
                            )
                for hp_z in range(2):
                    for jj_z in range(3):
                        nc.vector.tensor_copy(
                            kt[:, hp_z, jj_z, NV:768], zeros[:, 0:768 - NV])

                # ---------------- projection: V (per-rc shifted grid) ----------------
                xtv = xpool.tile([128, 8, NVV], F32R, tag="xt", name="xv")
                nc.sync.dma_start(xtv[:], xtv_d.rearrange("(c p) v -> p c v", p=128))
                for rc in range(3):
                    jjv = jv[rc]
                    delta = r0v[rc] - r0k[rc]
                    r0 = r0k[rc]
                    nrc = 683 if rc < 2 else 682
                    lim = r0 + nrc          # rows >= lim are invalid tokens
                    mlo, plo = divmod(lim, 128)
                    for m in range(NCHUNK):
                        if m > mlo or (m == mlo and plo == 0):
                            nc.vector.tensor_copy(yvs[:, m, rc, :], zeros[:])
                            continue
                        ps = psv.tile([128, 260], F32, tag="psv")
                        x0 = GUARD + 128 * m + delta
                        for ic in range(8):
                            nc.tensor.matmul(
                                ps[:],
                                xtv[:, ic, x0: x0 + 128],
                                wv[:, ic, jjv * 260:(jjv + 1) * 260],
                                start=(ic == 0),
                                stop=False,
                            )
                        nc.tensor.matmul(
                            ps[:],
                            ones[0:1, 0:128],
                            bv[0:1, jjv * 260:(jjv + 1) * 260],
                            start=False,
                            stop=True,
                        )
                        if m == mlo:
                            # tail chunk: zero all, then evict only valid rows
                            nc.vector.tensor_copy(yvs[:, m, rc, :], zeros[:])
                            nc.vector.tensor_copy(
                                yvs[0:plo, m, rc, :], ps[0:plo, :])
                        else:
                            nc.vector.tensor_copy(yvs[:, m, rc, :], ps[:])
                            if m == 0 and r0 > 0:
                                # head rows with token t < 0
                                nc.vector.tensor_copy(
                                    yvs[0:r0, 0, rc, :], zeros[0:r0, :])

                proj_ctx.close()
                attn_ctx = ExitStack()
                pss = attn_ctx.enter_context(
                    tc.tile_pool(name="pss", bufs=1, space="PSUM"))
                psctx = attn_ctx.enter_context(
                    tc.tile_pool(name="psctx", bufs=2, space="PSUM"))

                # ---------------- attention ----------------
                for hp in range(2):
                    for q0 in (0, QW):
                        chunks = _chunks(B2, q0)
                        # per (half): list of chunk indices contributing
                        half_valid = [
                            [i for i, (m, rc, t_min, a) in enumerate(chunks)
                             if t_min < q0 + 512 * (h + 1)]
                            for h in range(2)
                        ]
                        ctx = [
                            psctx.tile([65, QW], F32, tag="ctx", name=f"ctx{hr}")
                            for hr in range(2)
                        ]
                        for ci, (m, rc, t_min, a) in enumerate(chunks):
                            jjk, r0 = jk[rc], r0k[rc]
                            s_ps = pss.tile([128, 2048], F32, tag="s")
                            for hr in range(2):
                                pr = slice(hr * 64, hr * 64 + 64)
                                for h in range(2):
                                    if t_min >= q0 + 512 * (h + 1):
                                        continue
                                    ah = max(a, 512 * h)
                                    nc.tensor.matmul(
                                        s_ps[:, hr * 1024 + ah: hr * 1024 + 512 * (h + 1)],
                                        kt[pr, hp, jjk, 128 * m: 128 * (m + 1)],
                                        qt[pr, hp, q0 + ah: q0 + 512 * (h + 1)],
                                        start=True,
                                        stop=True,
                                        tile_position=(hr * 64, 0),
                                    )
                            p_sb = ppool.tile([128, 2, QW], F32R, tag="p")
                            s3 = s_ps[:].rearrange("p (h w) -> p h w", h=2)
                            nc.scalar.activation(
                                p_sb[:, :, a:QW],
                                s3[:, :, a:QW],
                                mybir.ActivationFunctionType.Exp,
                                scale=float(HS) ** -0.5,
                            )
                            ws, we = a, min(QW, t_min + 382 - q0)
                            if ws < we:
                                nc.gpsimd.affine_select(
                                    out=p_sb[:, :, ws:we],
                                    in_=p_sb[:, :, ws:we],
                                    pattern=[[0, 2], [1, we - ws]],
                                    compare_op=mybir.AluOpType.is_ge,
                                    fill=0.0,
                                    base=q0 + ws - rc - 384 * m + 3 * r0,
                                    channel_multiplier=-3,
                                )
                            for hr in range(2):
                                h_loc = 2 * hp + hr
                                for h in range(2):
                                    if t_min >= q0 + 512 * (h + 1):
                                        continue
                                    ah = max(a, 512 * h)
                                    nc.tensor.matmul(
                                        ctx[hr][:, ah: 512 * (h + 1)],
                                        yvs[:, m, rc, h_loc * 65:(h_loc + 1) * 65],
                                        p_sb[:, hr, ah: 512 * (h + 1)],
                                        start=(ci == half_valid[h][0]),
                                        stop=(ci == half_valid[h][-1]),
                                    )
                        # epilogue: divide by denominator, transpose, store
                        cx = cxpool.tile([65, 2, QW], F32, tag="cx")
                        for hr in range(2):
                            nc.vector.tensor_copy(cx[:, hr, :], ctx[hr][:])
                        o_sb = [opool.tile([128, 8, 64], F32, tag="o", name=f"o{hr}")
                                for hr in range(2)]
                        for hr in range(2):
                            for qb in range(QW // 128):
                                tr = pss.tile([128, 65], F32, tag="s", name="tr")
                                nc.tensor.transpose(
                                    tr[:],
                                    cx[0:65, hr, qb * 128:(qb + 1) * 128],
                                    identity[0:65, 0:65],
                                )
                                rec = rpool.tile([128, 1], F32, tag="rec")
                                nc.vector.reciprocal(rec[:], tr[:, 64:65])
                                nc.vector.tensor_scalar_mul(
                                    o_sb[hr][:, qb, :], tr[:, 0:64], rec[:]
                                )
                            nc.sync.dma_start(
                                out_d[q0: q0 + QW, (2 * hp + hr) * 64:
                                      (2 * hp + hr + 1) * 64].rearrange(
                                    "(qb p) d -> p qb d", p=128
                                ),
                                o_sb[hr][:],
                            )
                attn_ctx.close()

    nc.compile()
    return nc


# ---------------------------------------------------------------------------
# host-side data prep
# ---------------------------------------------------------------------------

def _perm_cols():
    perm = np.empty(3 * D, dtype=np.int64)
    for j in range(3):
        for h in range(NH):
            for d in range(HS):
                perm[j * D + h * HS + d] = j * D + d * NH + h
    return perm


def _core_inputs(xT, W2, b2, B2, HG):
    cst, jk, r0k, jv, r0v = _phase(B2)

    def xt_slice(c, ncols, guard=0):
        vs = cst[c]["vstart"] - guard
        sl = np.zeros((D, ncols), dtype=np.float32)
        lo, hi = max(0, vs), min(B * T, vs + ncols)
        sl[:, lo - vs: hi - vs] = xT[:, lo:hi]
        return sl

    WQK = np.empty((D, 768), dtype=np.float32)
    BQKf = np.empty(768, dtype=np.float32)
    for jj in range(3):
        src = jj * D + HG * 256
        WQK[:, jj * 256:(jj + 1) * 256] = W2[:, src:src + 256]
        BQKf[jj * 256:(jj + 1) * 256] = b2[src:src + 256]
    BQK = BQKf.reshape(6, 128).T.copy()  # [128, 6]: col fc, partition p

    WV = np.zeros((D, 780), dtype=np.float32)
    BV = np.zeros((1, 780), dtype=np.float32)
    for jj in range(3):
        for hl in range(4):
            src = jj * D + HG * 256 + hl * 64
            cb = (jj * 4 + hl) * 65
            WV[:, cb:cb + 64] = W2[:, src:src + 64]
            BV[0, cb:cb + 64] = b2[src:src + 64]
            BV[0, cb + 64] = 1.0

    return {
        "XTQ": xt_slice(0, 768),
        "XTK": xt_slice(1, 768),
        "XTV": xt_slice(2, NVV, guard=GUARD),
        "WQK": WQK,
        "WV": WV,
        "BQK": np.ascontiguousarray(BQK),
        "BV": BV,
        "ONES": np.ones((1, 128), dtype=np.float32),
        "ZEROS": np.zeros((128, 260), dtype=np.float32),
    }


# ---------------------------------------------------------------------------
# concurrent two-program dispatch (4+4 cores)
# ---------------------------------------------------------------------------

def _sharded_fn(nc, dev_lo, dev_hi):
    import jax
    from jax.sharding import Mesh, PartitionSpec
    from jax.experimental.shard_map import shard_map
    from concourse import bass2jax
    from concourse.bass2jax import _bass_exec_p, install_neuronx_cc_hook

    install_neuronx_cc_hook()
    n_cores = dev_hi - dev_lo

    in_names, out_names, out_avals, zero_shapes = [], [], [], []
    partition_name = (
        nc.partition_id_tensor.name if nc.partition_id_tensor else None
    )
    for alloc in nc.m.functions[0].allocations:
        if not isinstance(alloc, mybir.MemoryLocationSet):
            continue
        name = alloc.memorylocations[0].name
        if alloc.kind == "ExternalInput":
            if name != partition_name:
                in_names.append(name)
        elif alloc.kind == "ExternalOutput":
            np_dt = mybir.dt.np(alloc.dtype)
            out_avals.append(
                jax.core.ShapedArray(tuple(alloc.tensor_shape), np_dt)
            )
            out_names.append(name)
            zero_shapes.append((tuple(alloc.tensor_shape), np_dt))
    n_params = len(in_names)
    all_in_names = list(in_names) + list(out_names)
    if partition_name is not None:
        all_in_names.append(partition_name)

    donate = tuple(range(n_params, n_params + len(out_names)))

    def _body(*args):
        operands = list(args)
        if partition_name is not None:
            operands.append(bass2jax.partition_id_tensor())
        outs = _bass_exec_p.bind(
            *operands,
            out_avals=tuple(out_avals),
            in_names=tuple(all_in_names),
            out_names=tuple(out_names),
            lowering_input_output_aliases=(),
            sim_require_finite=True,
            sim_require_nnan=True,
            nc=nc,
        )
        return tuple(outs)

    devices = jax.devices()[dev_lo:dev_hi]
    mesh = Mesh(np.asarray(devices), ("core",))
    in_specs = (PartitionSpec("core"),) * (n_params + len(out_names))
    out_specs = (PartitionSpec("core"),) * len(out_names)
    fn = jax.jit(
        shard_map(_body, mesh=mesh, in_specs=in_specs, out_specs=out_specs,
                  check_rep=False),
        donate_argnums=donate,
        keep_unused=True,
    )
    return fn, in_names, out_names, out_avals, zero_shapes, n_cores


def _concat_inputs(in_maps, in_names):
    return [
        np.concatenate([np.asarray(m[name]) for m in in_maps], axis=0)
        for name in in_names
    ]


def kernel(x, W_qkv, b_qkv):
    x = np.asarray(x, dtype=np.float32)
    W_qkv = np.asarray(W_qkv, dtype=np.float32)
    b_qkv = np.asarray(b_qkv, dtype=np.float32)

    if "progs" not in _CACHE:
        _CACHE["progs"] = {
            B2: _build_program(B2, repeat=int(os.environ.get("KREPEAT", "1")))
            for B2 in range(2)
        }
        _CACHE["fns"] = {
            0: _sharded_fn(_CACHE["progs"][0], 0, 4),
            1: _sharded_fn(_CACHE["progs"][1], 4, 8),
        }

    perm = _perm_cols()
    W2 = W_qkv[:, perm]
    b2 = b_qkv[perm]
    xT = np.ascontiguousarray(x.reshape(B * T, D).T)

    results = {}
    pending = []
    for B2 in range(2):
        fn, in_names, out_names, out_avals, zero_shapes, n_cores = _CACHE["fns"][B2]
        in_maps = [_core_inputs(xT, W2, b2, B2, HG) for HG in range(4)]
        concat_in = _concat_inputs(in_maps, in_names)
        concat_zeros = [
            np.zeros((n_cores * s[0], *s[1:]), d) for (s, d) in zero_shapes
        ]
        out_arrs = fn(*concat_in, *concat_zeros)  # async dispatch
        pending.append((B2, out_names, out_avals, n_cores, out_arrs))

    out_full = np.zeros((B, T, D), dtype=np.float32)
    for B2, out_names, out_avals, n_cores, out_arrs in pending:
        per_core = np.asarray(out_arrs[0]).reshape(n_cores, T, 256)
        for HG in range(4):
            out_full[B2, :, HG * 256:(HG + 1) * 256] = per_core[HG]
    return out_full
